# revision 1
# baseline (speedup 1.0000x reference)
"""Trainium2 Bass kernel for the hypergraph-conv survival model.

Sharding: graph/data parallel over 8 NeuronCores. Core k owns graphs
2k,2k+1 (a contiguous node range, since `batch` is sorted). Both hconv
message-passing directions are sharded by that node range:
  phase A: scatter node rows -> per-core partial hyperedge sums
           (gather rows of the local z table by incidence, sorted by
            hyperedge; segment-sum via one-hot fp32r matmuls in PSUM)
  AllReduce the [25088,128] partial hyperedge table across cores
  phase B: gather reduced hyperedge rows by incidence sorted by node;
           segment-sum into the core's node windows.
Small weights are replicated; the final MLP runs per-core on its two
pooled graph rows.
"""

import sys

sys.path.insert(0, "/opt/trn_rl_repo")

import os
import numpy as np

DEBUG_STAGE = int(os.environ.get("KERNEL_DEBUG_STAGE", "0"))

# ---- static problem sizes (from the reference) ----
N = 100_000
E = 800_000
M = 25_000
B_GRAPHS = 16
F_IN = 64
H = 128
EPS = 1e-5
NCORE = 8

NK_PAD = 13312          # padded per-core node count (104*128; 52 windows of 256)
M_PAD = 25088           # padded hyperedge count  (196*128; 98 windows of 256)
W = 256                 # segment-sum window width
NWA = M_PAD // W        # 98 hedge windows (phase A)
NWB = NK_PAD // W       # 52 node windows (phase B)
TW_A = 10               # tiles (128 edges) per hedge window
TW_B = 19               # tiles per node window
E_PAD_A = NWA * TW_A * 128   # 125440
E_PAD_B = NWB * TW_B * 128   # 126464
CHW_A = 7               # windows per gather chunk, phase A (14 chunks)
CHW_B = 4               # windows per gather chunk, phase B (13 chunks)
NCH_A = NWA // CHW_A
NCH_B = NWB // CHW_B
NI_A = CHW_A * TW_A * 128    # idxs per phase-A gather (8960)
NI_B = CHW_B * TW_B * 128    # idxs per phase-B gather (9728)

_COMPILED = None


def _build_nc():
    import concourse.bacc as bacc
    import concourse.mybir as mybir
    from concourse.tile import TileContext
    from concourse import library_config

    f32 = mybir.dt.float32
    f32r = mybir.dt.float32r
    i16 = mybir.dt.int16
    i32 = mybir.dt.int32
    EQ = mybir.AluOpType.is_equal
    ADD = mybir.AluOpType.add
    MAX = mybir.AluOpType.max

    nc = bacc.Bacc("TRN2", target_bir_lowering=False, num_devices=NCORE)

    def inp(name, shape, dt=f32):
        return nc.dram_tensor(name, shape, dt, kind="ExternalInput")

    xT = inp("xT", [F_IN, NK_PAD])
    idxA = inp("idxA", [128, E_PAD_A // 16], i16)
    widxA = inp("widxA", [128, E_PAD_A // 128])
    idxB = inp("idxB", [128, E_PAD_B // 16], i16)
    widxB = inp("widxB", [128, E_PAD_B // 128])
    binv_c = inp("binv_c", [128, M_PAD // 128])
    dinv_r = inp("dinv_r", [128, NK_PAD])
    pool0_r = inp("pool0_r", [128, NK_PAD])
    pool1_r = inp("pool1_r", [128, NK_PAD])
    W0_d = inp("W0", [F_IN, H])
    Wc1_d = inp("Wc1", [H, H])
    Wc2_d = inp("Wc2", [H, H])
    WgA_d = inp("WgA", [H, H])
    WgB_d = inp("WgB", [H, H])
    W1_d = inp("W1f", [H, 64])
    W2_d = inp("W2f", [64, 32])
    W3_d = inp("W3", [32, 4])
    b0_d = inp("b0c", [H, 1])
    bc1_d = inp("bc1c", [H, 1])
    bc2_d = inp("bc2c", [H, 1])
    bg_d = inp("bgc", [H, 1])
    b1_d = inp("b1c", [64, 1])
    b2_d = inp("b2c", [32, 1])
    out_d = nc.dram_tensor("out", [4, 2], f32, kind="ExternalOutput")
    dbg_d = nc.dram_tensor("dbg", [128, H], f32, kind="ExternalOutput") if DEBUG_STAGE else None

    z1_h = nc.dram_tensor("z1_h", [NK_PAD, H], f32)
    z2_h = nc.dram_tensor("z2_h", [NK_PAD, H], f32)
    eA1 = nc.dram_tensor("eA1", [M_PAD, H], f32)
    eR1 = nc.dram_tensor("eR1", [M_PAD, H], f32, addr_space="Shared")
    eA2 = nc.dram_tensor("eA2", [M_PAD, H], f32)
    eR2 = nc.dram_tensor("eR2", [M_PAD, H], f32, addr_space="Shared")

    with TileContext(nc) as tc:
        with (
            tc.tile_pool(name="c", bufs=1) as cpool,
            tc.tile_pool(name="g", bufs=2) as gpool,
            tc.tile_pool(name="oh", bufs=4) as ohpool,
            tc.tile_pool(name="s", bufs=3) as spool,
            tc.tile_pool(name="ps", bufs=2, space="PSUM") as pspool,
            tc.tile_pool(name="acc", bufs=1) as accpool,
            tc.tile_pool(name="bc", bufs=2) as bcpool,
        ):
            nc.gpsimd.load_library(library_config.mlp)

            # ---- constants ----
            iota_i = cpool.tile([128, W], i32)
            nc.gpsimd.iota(iota_i[:], [[1, W]], channel_multiplier=0)
            iota_f = cpool.tile([128, W], f32)
            nc.vector.tensor_copy(iota_f[:], iota_i[:])
            idn_i = cpool.tile([128, 128], i32)
            nc.gpsimd.iota(idn_i[:], [[1, 128]], channel_multiplier=-1)
            ident = cpool.tile([128, 128], f32)
            nc.vector.tensor_scalar(ident[:], idn_i[:], 0.0, None, EQ)

            def load_sb(dram, shape, dt=f32):
                t = cpool.tile(shape, dt, tag=dram.name + "_sb")
                nc.sync.dma_start(out=t[:], in_=dram[:, :])
                return t

            W0s = load_sb(W0_d, [F_IN, H])
            Wc1s = load_sb(Wc1_d, [H, H])
            Wc2s = load_sb(Wc2_d, [H, H])
            WgAs = load_sb(WgA_d, [H, H])
            WgBs = load_sb(WgB_d, [H, H])
            W1s = load_sb(W1_d, [H, 64])
            W2s = load_sb(W2_d, [64, 32])
            W3s = load_sb(W3_d, [32, 4])
            b0s = load_sb(b0_d, [H, 1])
            bc1s = load_sb(bc1_d, [H, 1])
            bc2s = load_sb(bc2_d, [H, 1])
            bgs = load_sb(bg_d, [H, 1])
            b1s = load_sb(b1_d, [64, 1])
            b2s = load_sb(b2_d, [32, 1])
            binvs = load_sb(binv_c, [128, M_PAD // 128])
            idxAs = load_sb(idxA, [128, E_PAD_A // 16], i16)
            widxAs = load_sb(widxA, [128, E_PAD_A // 128])
            idxBs = load_sb(idxB, [128, E_PAD_B // 16], i16)
            widxBs = load_sb(widxB, [128, E_PAD_B // 128])

            p1acc = accpool.tile([128, 2], f32)
            nc.vector.memset(p1acc[:], 0.0)
            p2acc = accpool.tile([128, 2], f32)
            nc.vector.memset(p2acc[:], 0.0)

            def wr_rows(src_sb, dst_hbm, row0):
                """Transpose [128feat, 256] chunk to rows and DMA out,
                optional per-row scale col applied by caller beforehand."""
                for h_ in range(2):
                    tp = pspool.tile([128, 128], f32, tag="trp")
                    nc.tensor.transpose(tp[:], src_sb[:, h_ * 128:(h_ + 1) * 128], ident[:])
                    yield tp, dst_hbm, row0 + h_ * 128

            # ---- stage 1+2: z1 = (relu(x@W0+b0)) @ Wc1, node-major to HBM ----
            for c in range(NWB):
                xc = spool.tile([F_IN, W], f32, tag="xc")
                nc.sync.dma_start(out=xc[:], in_=xT[:, c * W:(c + 1) * W])
                ps1 = pspool.tile([128, W], f32, tag="st")
                nc.tensor.matmul(ps1[:], W0s[:], xc[:], start=True, stop=True)
                h1c = spool.tile([128, W], f32, tag="h1c")
                nc.vector.tensor_scalar(h1c[:], ps1[:], b0s[:], 0.0, ADD, MAX)
                ps2 = pspool.tile([128, W], f32, tag="st")
                nc.tensor.matmul(ps2[:], Wc1s[:], h1c[:], start=True, stop=True)
                zc = spool.tile([128, W], f32, tag="edc")
                nc.vector.tensor_copy(zc[:], ps2[:])
                for tp, dst, r0 in wr_rows(zc, z1_h, c * W):
                    rows = spool.tile([128, 128], f32, tag="rows")
                    nc.vector.tensor_copy(rows[:], tp[:])
                    nc.sync.dma_start(out=dst[r0:r0 + 128, :], in_=rows[:])

            # ---- conv phase A: partial hyperedge sums ----
            def phase_A(table_h, out_h):
                for ch in range(NCH_A):
                    g = gpool.tile([128, CHW_A * TW_A, 128], f32r, tag="g")
                    nc.gpsimd.dma_gather(
                        g[:], table_h.ap().bitcast(f32r),
                        idxAs[:, ch * (NI_A // 16):(ch + 1) * (NI_A // 16)],
                        NI_A, NI_A, 128, single_packet=False,
                    )
                    for wi in range(CHW_A):
                        w = ch * CHW_A + wi
                        ps = pspool.tile([128, W], f32, tag="phps")
                        for tt in range(TW_A):
                            tg = w * TW_A + tt
                            oh = ohpool.tile([128, W], f32r, tag="oh")
                            nc.vector.tensor_scalar(
                                oh[:], iota_f[:], widxAs[:, tg:tg + 1], None, EQ)
                            nc.tensor.matmul(
                                ps[:], g[:, wi * TW_A + tt, :], oh[:],
                                start=(tt == 0), stop=(tt == TW_A - 1))
                        edc = spool.tile([128, W], f32, tag="edc")
                        nc.vector.tensor_copy(edc[:], ps[:])
                        for tp, dst, r0 in wr_rows(edc, out_h, w * W):
                            rows = spool.tile([128, 128], f32, tag="rows")
                            nc.vector.tensor_scalar_mul(
                                rows[:], tp[:], binvs[:, r0 // 128:r0 // 128 + 1])
                            nc.sync.dma_start(out=dst[r0:r0 + 128, :], in_=rows[:])

            # ---- conv phase B: node sums + relu + pooling (+ next z) ----
            def phase_B(table_h, bias_s, pacc, Wnext, znext_h):
                for ch in range(NCH_B):
                    g = gpool.tile([128, CHW_B * TW_B, 128], f32r, tag="g")
                    nc.gpsimd.dma_gather(
                        g[:], table_h.ap().bitcast(f32r),
                        idxBs[:, ch * (NI_B // 16):(ch + 1) * (NI_B // 16)],
                        NI_B, NI_B, 128, single_packet=False,
                    )
                    csl = slice(ch * CHW_B * W, (ch + 1) * CHW_B * W)
                    dch = bcpool.tile([128, CHW_B * W], f32, tag="dch")
                    nc.sync.dma_start(out=dch[:], in_=dinv_r[:, csl])
                    p0ch = bcpool.tile([128, CHW_B * W], f32, tag="p0ch")
                    nc.sync.dma_start(out=p0ch[:], in_=pool0_r[:, csl])
                    p1ch = bcpool.tile([128, CHW_B * W], f32, tag="p1ch")
                    nc.sync.dma_start(out=p1ch[:], in_=pool1_r[:, csl])
                    for wi in range(CHW_B):
                        w = ch * CHW_B + wi
                        ps = pspool.tile([128, W], f32, tag="phps")
                        for tt in range(TW_B):
                            tg = w * TW_B + tt
                            oh = ohpool.tile([128, W], f32r, tag="oh")
                            nc.vector.tensor_scalar(
                                oh[:], iota_f[:], widxBs[:, tg:tg + 1], None, EQ)
                            nc.tensor.matmul(
                                ps[:], g[:, wi * TW_B + tt, :], oh[:],
                                start=(tt == 0), stop=(tt == TW_B - 1))
                        sl = slice(wi * W, (wi + 1) * W)
                        tmp = spool.tile([128, W], f32, tag="tmp")
                        nc.vector.tensor_mul(
                            tmp[:], ps[:], dch[:, sl])
                        h2c = spool.tile([128, W], f32, tag="h2c")
                        nc.vector.tensor_scalar(h2c[:], tmp[:], bias_s[:], 0.0, ADD, MAX)
                        for gi, prow in ((0, p0ch), (1, p1ch)):
                            pm = spool.tile([128, W], f32, tag="pm")
                            nc.vector.tensor_mul(
                                pm[:], h2c[:], prow[:, sl])
                            rs = spool.tile([128, 1], f32, tag="rs")
                            nc.vector.tensor_reduce(
                                rs[:], pm[:], mybir.AxisListType.X, ADD)
                            nc.vector.tensor_add(
                                pacc[:, gi:gi + 1], pacc[:, gi:gi + 1], rs[:])
                        if znext_h is not None:
                            zp = pspool.tile([128, W], f32, tag="st")
                            nc.tensor.matmul(zp[:], Wnext[:], h2c[:], start=True, stop=True)
                            zc = spool.tile([128, W], f32, tag="edc")
                            nc.vector.tensor_copy(zc[:], zp[:])
                            for tp, dst, r0 in wr_rows(zc, znext_h, w * W):
                                rows = spool.tile([128, 128], f32, tag="rows")
                                nc.vector.tensor_copy(rows[:], tp[:])
                                nc.sync.dma_start(out=dst[r0:r0 + 128, :], in_=rows[:])

            phase_A(z1_h, eA1)
            nc.gpsimd.collective_compute(
                "AllReduce", ADD, replica_groups=[list(range(NCORE))],
                ins=[eA1.ap().opt()], outs=[eR1.ap().opt()])
            if DEBUG_STAGE == 1:
                dt_ = spool.tile([128, H], f32, tag="dbgt")
                nc.sync.dma_start(out=dt_[:], in_=eR1[0:128, :])
                nc.sync.dma_start(out=dbg_d[:, :], in_=dt_[:])
            if not DEBUG_STAGE:
                phase_B(eR1, bc1s, p1acc, Wc2s, z2_h)
                phase_A(z2_h, eA2)
                nc.gpsimd.collective_compute(
                    "AllReduce", ADD, replica_groups=[list(range(NCORE))],
                    ins=[eA2.ap().opt()], outs=[eR2.ap().opt()])
                phase_B(eR2, bc2s, p2acc, None, None)

            # ---- MLP head on the two pooled rows ----
            gps = pspool.tile([128, 2], f32, tag="mlp")
            nc.tensor.matmul(gps[:], WgAs[:], p1acc[:], start=True, stop=False)
            nc.tensor.matmul(gps[:], WgBs[:], p2acc[:], start=False, stop=True)
            gb = spool.tile([128, 2], f32, tag="m1")
            nc.vector.tensor_scalar(gb[:], gps[:], bgs[:], None, ADD)
            h1ps = pspool.tile([64, 2], f32, tag="mlp")
            nc.tensor.matmul(h1ps[:], W1s[:], gb[:], start=True, stop=True)
            h1m = spool.tile([64, 2], f32, tag="m2")
            nc.vector.tensor_scalar(h1m[:], h1ps[:], b1s[:], 0.0, ADD, MAX)
            h2ps = pspool.tile([32, 2], f32, tag="mlp")
            nc.tensor.matmul(h2ps[:], W2s[:], h1m[:], start=True, stop=True)
            h2m = spool.tile([32, 2], f32, tag="m3")
            nc.vector.tensor_scalar(h2m[:], h2ps[:], b2s[:], 0.0, ADD, MAX)
            ops = pspool.tile([4, 2], f32, tag="mlp")
            nc.tensor.matmul(ops[:], W3s[:], h2m[:], start=True, stop=True)
            om = spool.tile([4, 2], f32, tag="m4")
            nc.vector.tensor_copy(om[:], ops[:])
            nc.sync.dma_start(out=out_d[:, :], in_=om[:])

    nc.compile()
    return nc


def _wrap_idx(idx):
    return np.tile(idx.reshape(-1, 16).T, (8, 1)).copy()


def _prep_core(k, x, node_idx, hedge_idx, batch):
    s = int(np.searchsorted(batch, 2 * k))
    e = int(np.searchsorted(batch, 2 * k + 2))
    nk = e - s
    assert nk <= NK_PAD
    sel = np.where((node_idx >= s) & (node_idx < e))[0]
    na = (node_idx[sel] - s).astype(np.int64)
    ha = hedge_idx[sel].astype(np.int64)

    def build(keys, vals, nwin, tw):
        order = np.argsort(keys, kind="stable")
        ks, vs = keys[order], vals[order]
        gidx = np.zeros(nwin * tw * 128, np.int16)
        widx = np.full(nwin * tw * 128, -1.0, np.float32)
        starts = np.searchsorted(ks, np.arange(nwin) * W)
        ends = np.searchsorted(ks, (np.arange(nwin) + 1) * W)
        for w in range(nwin):
            a, b = starts[w], ends[w]
            n = b - a
            assert n <= tw * 128, f"window overflow {n} > {tw * 128}"
            o = w * tw * 128
            gidx[o:o + n] = vs[a:b]
            widx[o:o + n] = ks[a:b] - w * W
        return _wrap_idx(gidx), widx.reshape(-1, 128).T.copy()

    giA, wiA = build(ha, na, NWA, TW_A)        # scatter by hedge; gather z rows
    giB, wiB = build(na, ha, NWB, TW_B)        # gather edge rows; by dest node

    xT = np.zeros((F_IN, NK_PAD), np.float32)
    xT[:, :nk] = x[s:e].T

    deg = np.bincount(node_idx, minlength=N).astype(np.float32)
    dloc = np.zeros(NK_PAD, np.float32)
    dl = deg[s:e]
    dloc[:nk] = np.where(dl > 0, 1.0 / np.maximum(dl, 1), 0.0)

    p0 = np.zeros(NK_PAD, np.float32)
    p1 = np.zeros(NK_PAD, np.float32)
    bloc = batch[s:e]
    for gi, arr in ((2 * k, p0), (2 * k + 1, p1)):
        m = (bloc == gi)
        cnt = max(float(m.sum()), 1.0)
        arr[:nk][m] = 1.0 / cnt
    return {
        "xT": xT, "idxA": giA, "widxA": wiA, "idxB": giB, "widxB": wiB,
        "dinv_r": np.ascontiguousarray(np.broadcast_to(dloc, (128, NK_PAD))),
        "pool0_r": np.ascontiguousarray(np.broadcast_to(p0, (128, NK_PAD))),
        "pool1_r": np.ascontiguousarray(np.broadcast_to(p1, (128, NK_PAD))),
    }


def kernel(x, node_idx, hedge_idx, batch, W0, b0, Wc1, bc1, Wc2, bc2,
           Wg, bg, W1, b1, g1, be1, rm1, rv1, W2, b2, g2, be2, rm2, rv2, W3):
    global _COMPILED
    from concourse.bass_utils import run_bass_kernel_spmd

    x = np.asarray(x, np.float32)
    node_idx = np.asarray(node_idx).astype(np.int64)
    hedge_idx = np.asarray(hedge_idx).astype(np.int64)
    batch_np = np.asarray(batch).astype(np.int64)

    if _COMPILED is None:
        _COMPILED = _build_nc()
    nc = _COMPILED

    # replicated (weight) inputs, with eval-BN folded into W1/W2
    k1 = np.asarray(g1) / np.sqrt(np.asarray(rv1) + EPS)
    W1f = (np.asarray(W1) * k1[None, :]).astype(np.float32)
    b1f = ((np.asarray(b1) - np.asarray(rm1)) * k1 + np.asarray(be1)).astype(np.float32)
    k2 = np.asarray(g2) / np.sqrt(np.asarray(rv2) + EPS)
    W2f = (np.asarray(W2) * k2[None, :]).astype(np.float32)
    b2f = ((np.asarray(b2) - np.asarray(rm2)) * k2 + np.asarray(be2)).astype(np.float32)

    cnt = np.bincount(hedge_idx, minlength=M_PAD).astype(np.float32)
    binv = np.where(cnt > 0, 1.0 / np.maximum(cnt, 1), 0.0).astype(np.float32)

    Wg_np = np.asarray(Wg, np.float32)
    shared = {
        "binv_c": binv.reshape(-1, 128).T.copy(),
        "W0": np.asarray(W0, np.float32), "Wc1": np.asarray(Wc1, np.float32),
        "Wc2": np.asarray(Wc2, np.float32),
        "WgA": Wg_np[:H], "WgB": Wg_np[H:],
        "W1f": W1f, "W2f": W2f, "W3": np.asarray(W3, np.float32),
        "b0c": np.asarray(b0, np.float32).reshape(-1, 1),
        "bc1c": np.asarray(bc1, np.float32).reshape(-1, 1),
        "bc2c": np.asarray(bc2, np.float32).reshape(-1, 1),
        "bgc": np.asarray(bg, np.float32).reshape(-1, 1),
        "b1c": b1f.reshape(-1, 1), "b2c": b2f.reshape(-1, 1),
    }
    in_maps = []
    for k in range(NCORE):
        m = _prep_core(k, x, node_idx, hedge_idx, batch_np)
        m.update(shared)
        in_maps.append(m)

    r = run_bass_kernel_spmd(nc, in_maps, core_ids=list(range(NCORE)))
    out = np.zeros((B_GRAPHS, 4), np.float32)
    for k in range(NCORE):
        o = r.results[k]["out"]
        out[2 * k] = o[:, 0]
        out[2 * k + 1] = o[:, 1]
    return out



# revision 14
# speedup vs baseline: 1.9053x; 1.9053x over previous
"""Trainium2 Bass kernel for the hypergraph-conv survival model.

Graph/data parallel over 8 NeuronCores; core k owns graphs 2k,2k+1.
Both hconv directions are segment-sums done as one-hot matmuls; the
one-hot tiles are PRECOMPUTED ON THE HOST (bf16) and streamed from HBM
(they only depend on the static incidence lists), so no engine has to
generate them. Gather tables (z1/z2/eR) are bf16 to halve gather bytes.
Phase A uses 512-wide hedge windows (psum [128,512] f32, one bank);
phase B keeps 256-wide node windows feature-major for pooling + the
next-layer matmul. The AllReduce stays f32; a SWDGE cast-DMA converts
the reduced table to bf16 for the phase-B gathers. PSUM evacuation
copies run on the otherwise-idle Activation engine.
"""

import sys

sys.path.insert(0, "/opt/trn_rl_repo")

import numpy as np
import ml_dtypes

BF16 = ml_dtypes.bfloat16

# ---- static problem sizes (from the reference) ----
N = 100_000
E = 800_000
M = 25_000
B_GRAPHS = 16
F_IN = 64
H = 128
EPS = 1e-5
NCORE = 8

NK_PAD = 13312          # padded per-core node count (104*128)
M_PAD = 25088           # padded hyperedge count  (196*128)

WA = 512                # phase-A (hedge) window width
NWA = M_PAD // WA       # 49 windows
TW_A = 18               # tiles (128 edges) per hedge window (data max 2194)
E_PAD_A = NWA * TW_A * 128   # 112896
CHW_A = 3               # windows per gather chunk (last chunk = 1 window)

WB = 256                # phase-B (node) window width
NWB = NK_PAD // WB      # 52 windows
TW_B = 18               # tiles per node window (data max 2201)
E_PAD_B = NWB * TW_B * 128   # 119808
CHW_B = 4               # windows per gather chunk (13 chunks)
NCH_B = NWB // CHW_B

A_CHUNKS = [(i, CHW_A) for i in range(0, NWA - 1, CHW_A)] + [(NWA - 1, 1)]

_COMPILED = None


def _build_nc():
    import concourse.bacc as bacc
    import concourse.mybir as mybir
    from concourse.tile import TileContext
    from concourse import library_config

    f32 = mybir.dt.float32
    bf16 = mybir.dt.bfloat16
    i16 = mybir.dt.int16
    i32 = mybir.dt.int32
    EQ = mybir.AluOpType.is_equal
    ADD = mybir.AluOpType.add
    MAX = mybir.AluOpType.max
    RELU = mybir.ActivationFunctionType.Relu

    nc = bacc.Bacc("TRN2", target_bir_lowering=False, num_devices=NCORE,
                   num_swdge_queues=2)

    def inp(name, shape, dt=f32):
        return nc.dram_tensor(name, shape, dt, kind="ExternalInput")

    xT = inp("xT", [F_IN, NK_PAD])
    idxA = inp("idxA", [128, E_PAD_A // 16], i16)
    idxB = inp("idxB", [128, E_PAD_B // 16], i16)
    ohA_d = inp("ohA", [128, NWA * TW_A * WA], bf16)
    ohB_d = inp("ohB", [128, NWB * TW_B * WB], bf16)
    binv_c = inp("binv_c", [128, M_PAD // 128])
    pool0_r = inp("pool0_r", [128, NK_PAD], bf16)
    pool1_r = inp("pool1_r", [128, NK_PAD], bf16)
    W0_d = inp("W0", [F_IN, H])
    Wc1_d = inp("Wc1", [H, H])
    Wc2_d = inp("Wc2b", [H, H], bf16)
    WgA_d = inp("WgA", [H, H])
    WgB_d = inp("WgB", [H, H])
    W1_d = inp("W1f", [H, 64])
    W2_d = inp("W2f", [64, 32])
    W3_d = inp("W3", [32, 4])
    b0_d = inp("b0c", [H, 1])
    bc1_d = inp("bc1c", [H, 1])
    bc2_d = inp("bc2c", [H, 1])
    bg_d = inp("bgc", [H, 1])
    b1_d = inp("b1c", [64, 1])
    b2_d = inp("b2c", [32, 1])
    out_d = nc.dram_tensor("out", [4, 2], f32, kind="ExternalOutput")

    z1_h = nc.dram_tensor("z1_h", [NK_PAD, H], bf16)
    z2_h = nc.dram_tensor("z2_h", [NK_PAD, H], bf16)
    eA1 = nc.dram_tensor("eA1", [M_PAD, H], bf16)
    eR1 = nc.dram_tensor("eR1", [M_PAD, H], bf16, addr_space="Shared")
    eA2 = nc.dram_tensor("eA2", [M_PAD, H], bf16)
    eR2 = nc.dram_tensor("eR2", [M_PAD, H], bf16, addr_space="Shared")

    with TileContext(nc) as tc:
        with (
            tc.tile_pool(name="c", bufs=1) as cpool,
            tc.tile_pool(name="ga", bufs=2) as gapool,
            tc.tile_pool(name="gb", bufs=3) as gbpool,
            tc.tile_pool(name="oha", bufs=2) as ohapool,
            tc.tile_pool(name="ohb", bufs=2) as ohbpool,
            tc.tile_pool(name="s", bufs=3) as spool,
            tc.tile_pool(name="psA", bufs=2, space="PSUM") as psApool,
            tc.tile_pool(name="psB", bufs=2, space="PSUM") as psBpool,
            tc.tile_pool(name="psz", bufs=2, space="PSUM") as pszpool,
            tc.tile_pool(name="pst", bufs=2, space="PSUM") as pstpool,
            tc.tile_pool(name="acc", bufs=1) as accpool,
            tc.tile_pool(name="bc", bufs=1) as bcpool,
        ):
            nc.gpsimd.load_library(library_config.mlp)

            idn_i = cpool.tile([128, 128], i32)
            nc.gpsimd.iota(idn_i[:], [[1, 128]], channel_multiplier=-1)
            ident = cpool.tile([128, 128], f32)
            nc.vector.tensor_scalar(ident[:], idn_i[:], 0.0, None, EQ)

            def load_sb(dram, shape, dt=f32):
                t = cpool.tile(shape, dt, tag=dram.name + "_sb")
                nc.sync.dma_start(out=t[:], in_=dram[:, :])
                return t

            W0s = load_sb(W0_d, [F_IN, H])
            Wc1s = load_sb(Wc1_d, [H, H])
            Wc2s = load_sb(Wc2_d, [H, H], bf16)
            WgAs = load_sb(WgA_d, [H, H])
            WgBs = load_sb(WgB_d, [H, H])
            W1s = load_sb(W1_d, [H, 64])
            W2s = load_sb(W2_d, [64, 32])
            W3s = load_sb(W3_d, [32, 4])
            b0s = load_sb(b0_d, [H, 1])
            bc1s = load_sb(bc1_d, [H, 1])
            bc2s = load_sb(bc2_d, [H, 1])
            bgs = load_sb(bg_d, [H, 1])
            b1s = load_sb(b1_d, [64, 1])
            b2s = load_sb(b2_d, [32, 1])
            binvs = load_sb(binv_c, [128, M_PAD // 128])
            idxAs = load_sb(idxA, [128, E_PAD_A // 16], i16)
            idxBs = load_sb(idxB, [128, E_PAD_B // 16], i16)

            p1acc = accpool.tile([128, 2], f32)
            nc.vector.memset(p1acc[:], 0.0)
            p2acc = accpool.tile([128, 2], f32)
            nc.vector.memset(p2acc[:], 0.0)

            # ---- stage 1+2: z1 = (relu(x@W0+b0)) @ Wc1, bf16 rows to HBM ----
            for c in range(NK_PAD // 512):
                xc = spool.tile([F_IN, 512], f32, tag="xc")
                nc.sync.dma_start(out=xc[:], in_=xT[:, c * 512:(c + 1) * 512])
                ps1 = pszpool.tile([128, 512], f32, tag="st1")
                nc.tensor.matmul(ps1[:], W0s[:], xc[:], start=True, stop=True)
                h1c = spool.tile([128, 512], f32, tag="h1c")
                nc.scalar.activation(h1c[:], ps1[:], RELU, bias=b0s[:, 0:1])
                ps2 = pszpool.tile([128, 512], f32, tag="st1")
                nc.tensor.matmul(ps2[:], Wc1s[:], h1c[:], start=True, stop=True)
                zc = spool.tile([128, 512], f32, tag="zc1")
                nc.scalar.copy(zc[:], ps2[:])
                for h_ in range(4):
                    tp = pstpool.tile([128, 128], f32, tag="trp")
                    nc.tensor.transpose(tp[:], zc[:, h_ * 128:(h_ + 1) * 128], ident[:])
                    rows = spool.tile([128, 128], bf16, tag="rowsz")
                    nc.scalar.copy(rows[:], tp[:])
                    nc.scalar.dma_start(
                        out=z1_h[c * 512 + h_ * 128:c * 512 + (h_ + 1) * 128, :],
                        in_=rows[:])

            gq = [0]

            def next_q():
                gq[0] ^= 1
                return gq[0]

            # ---- conv phase A: partial hyperedge sums (512-wide windows) ----
            def phase_A(table_h, out_h):
                for w0, nw in A_CHUNKS:
                    ni = nw * TW_A * 128
                    g = gapool.tile([128, CHW_A * TW_A, 128], bf16, tag="gA")
                    o16 = w0 * TW_A * 128 // 16
                    nc.gpsimd.dma_gather(
                        g[:, :nw * TW_A, :], table_h.ap(),
                        idxAs[:, o16:o16 + ni // 16],
                        ni, ni, 128, single_packet=False, queue_num=next_q(),
                    )
                    for wi in range(nw):
                        w = w0 + wi
                        ohw = ohapool.tile([128, TW_A, WA], bf16, tag="ohA")
                        oc = w * TW_A * WA
                        nc.sync.dma_start(out=ohw[:], in_=ohA_d[:, oc:oc + TW_A * WA])
                        ps = psApool.tile([128, WA], f32, tag="pA")
                        for tt in range(TW_A):
                            nc.tensor.matmul(
                                ps[:], g[:, wi * TW_A + tt, :], ohw[:, tt, :],
                                start=(tt == 0), stop=(tt == TW_A - 1))
                        edc = spool.tile([128, WA], f32, tag="edcA")
                        nc.scalar.copy(edc[:], ps[:])
                        for j in range(WA // 128):
                            blk = w * (WA // 128) + j
                            tp = pstpool.tile([128, 128], f32, tag="trp")
                            nc.tensor.transpose(
                                tp[:], edc[:, j * 128:(j + 1) * 128], ident[:])
                            rows = spool.tile([128, 128], bf16, tag="rowsA")
                            nc.scalar.mul(rows[:], tp[:], binvs[:, blk:blk + 1])
                            nc.sync.dma_start(
                                out=out_h[blk * 128:(blk + 1) * 128, :], in_=rows[:])

            # ---- conv phase B: node sums + relu + pooling (+ next z) ----
            def phase_B(table_h, bias_s, pacc, Wnext, znext_h):
                for ch in range(NCH_B):
                    ni = CHW_B * TW_B * 128
                    g = gbpool.tile([128, CHW_B * TW_B, 128], bf16, tag="gB")
                    o16 = ch * ni // 16
                    nc.gpsimd.dma_gather(
                        g[:], table_h.ap(),
                        idxBs[:, o16:o16 + ni // 16],
                        ni, ni, 128, single_packet=False, queue_num=next_q(),
                    )
                    csl = slice(ch * CHW_B * WB, (ch + 1) * CHW_B * WB)
                    p0ch = bcpool.tile([128, CHW_B * WB], bf16, tag="p0ch")
                    nc.sync.dma_start(out=p0ch[:], in_=pool0_r[:, csl])
                    p1ch = bcpool.tile([128, CHW_B * WB], bf16, tag="p1ch")
                    nc.sync.dma_start(out=p1ch[:], in_=pool1_r[:, csl])
                    for wi in range(CHW_B):
                        w = ch * CHW_B + wi
                        ohw = ohbpool.tile([128, TW_B, WB], bf16, tag="ohB")
                        oc = w * TW_B * WB
                        nc.sync.dma_start(out=ohw[:], in_=ohB_d[:, oc:oc + TW_B * WB])
                        ps = psBpool.tile([128, WB], f32, tag="pB")
                        for tt in range(TW_B):
                            nc.tensor.matmul(
                                ps[:], g[:, wi * TW_B + tt, :], ohw[:, tt, :],
                                start=(tt == 0), stop=(tt == TW_B - 1))
                        sl = slice(wi * WB, (wi + 1) * WB)
                        h2c = spool.tile([128, WB], bf16, tag="h2c")
                        nc.vector.tensor_scalar(h2c[:], ps[:], bias_s[:], 0.0, ADD, MAX)
                        for gi, prow in ((0, p0ch), (1, p1ch)):
                            pm = spool.tile([128, WB], bf16, tag="pm")
                            nc.vector.tensor_mul(pm[:], h2c[:], prow[:, sl])
                            rs = spool.tile([128, 1], f32, tag="rs")
                            nc.vector.tensor_reduce(
                                rs[:], pm[:], mybir.AxisListType.X, ADD)
                            nc.vector.tensor_add(
                                pacc[:, gi:gi + 1], pacc[:, gi:gi + 1], rs[:])
                        if znext_h is not None:
                            zp_t = pszpool.tile([128, 512], f32, tag="st1")
                            zp = zp_t[:, 0:WB]
                            nc.tensor.matmul(zp, Wnext[:], h2c[:], start=True, stop=True)
                            zc = spool.tile([128, WB], f32, tag="zc")
                            nc.scalar.copy(zc[:], zp)
                            for h_ in range(2):
                                tp = pstpool.tile([128, 128], f32, tag="trp")
                                nc.tensor.transpose(
                                    tp[:], zc[:, h_ * 128:(h_ + 1) * 128], ident[:])
                                rows = spool.tile([128, 128], bf16, tag="rowsz")
                                nc.scalar.copy(rows[:], tp[:])
                                nc.scalar.dma_start(
                                    out=znext_h[w * WB + h_ * 128:w * WB + (h_ + 1) * 128, :],
                                    in_=rows[:])

            phase_A(z1_h, eA1)
            HM = M_PAD // 2
            nc.gpsimd.collective_compute(
                "AllReduce", ADD, replica_groups=[list(range(NCORE))],
                ins=[eA1[0:HM, :].opt()], outs=[eR1[0:HM, :].opt()])
            nc.gpsimd.collective_compute(
                "AllReduce", ADD, replica_groups=[list(range(NCORE))],
                ins=[eA1[HM:M_PAD, :].opt()], outs=[eR1[HM:M_PAD, :].opt()])
            phase_B(eR1, bc1s, p1acc, Wc2s, z2_h)
            phase_A(z2_h, eA2)
            nc.gpsimd.collective_compute(
                "AllReduce", ADD, replica_groups=[list(range(NCORE))],
                ins=[eA2[0:HM, :].opt()], outs=[eR2[0:HM, :].opt()])
            nc.gpsimd.collective_compute(
                "AllReduce", ADD, replica_groups=[list(range(NCORE))],
                ins=[eA2[HM:M_PAD, :].opt()], outs=[eR2[HM:M_PAD, :].opt()])
            phase_B(eR2, bc2s, p2acc, None, None)

            # ---- MLP head on the two pooled rows ----
            gps_t = psBpool.tile([128, WB], f32, tag="pB")
            gps = gps_t[:, 0:2]
            nc.tensor.matmul(gps, WgAs[:], p1acc[:], start=True, stop=False)
            nc.tensor.matmul(gps, WgBs[:], p2acc[:], start=False, stop=True)
            gb = spool.tile([128, 2], f32, tag="m1")
            nc.vector.tensor_scalar(gb[:], gps, bgs[:], None, ADD)
            h1ps = mlppool.tile([64, 2], f32, tag="mlp2")
            nc.tensor.matmul(h1ps[:], W1s[:], gb[:], start=True, stop=True)
            h1m = spool.tile([64, 2], f32, tag="m2")
            nc.vector.tensor_scalar(h1m[:], h1ps[:], b1s[:], 0.0, ADD, MAX)
            h2ps = mlppool.tile([32, 2], f32, tag="mlp3")
            nc.tensor.matmul(h2ps[:], W2s[:], h1m[:], start=True, stop=True)
            h2m = spool.tile([32, 2], f32, tag="m3")
            nc.vector.tensor_scalar(h2m[:], h2ps[:], b2s[:], 0.0, ADD, MAX)
            ops = mlppool.tile([4, 2], f32, tag="mlp4")
            nc.tensor.matmul(ops[:], W3s[:], h2m[:], start=True, stop=True)
            om = spool.tile([4, 2], f32, tag="m4")
            nc.vector.tensor_copy(om[:], ops[:])
            nc.sync.dma_start(out=out_d[:, :], in_=om[:])

    nc.compile()
    return nc


def _wrap_idx(idx):
    return np.tile(idx.reshape(-1, 16).T, (8, 1)).copy()


def _build_phase(keys, vals, nwin, tw, W, slot_scale=None):
    """Sorted segment layout: window-major slot list, idx + one-hot tiles.
    One-hot entries are 1.0, or slot_scale[key] when given (folds the
    per-destination normalization into the segment-sum matmul)."""
    order = np.argsort(keys, kind="stable")
    ks, vs = keys[order], vals[order]
    gidx = np.zeros(nwin * tw * 128, np.int64)
    widx = np.full(nwin * tw * 128, -1.0, np.float32)
    sval = np.zeros(nwin * tw * 128, np.float32)
    starts = np.searchsorted(ks, np.arange(nwin) * W)
    ends = np.searchsorted(ks, (np.arange(nwin) + 1) * W)
    scale = np.ones(len(ks), np.float32) if slot_scale is None \
        else slot_scale[ks].astype(np.float32)
    for w in range(nwin):
        a, b = starts[w], ends[w]
        n = b - a
        assert n <= tw * 128, f"window overflow {n} > {tw * 128}"
        o = w * tw * 128
        gidx[o:o + n] = vs[a:b]
        widx[o:o + n] = ks[a:b] - w * W
        sval[o:o + n] = scale[a:b]
    oh = (widx.reshape(nwin * tw, 128)[:, :, None] ==
          np.arange(W, dtype=np.float32)[None, None, :])
    oh = oh * sval.reshape(nwin * tw, 128)[:, :, None]
    oh = np.ascontiguousarray(
        oh.transpose(1, 0, 2).reshape(128, nwin * tw * W)).astype(BF16)
    return _wrap_idx(gidx.astype(np.int16)), oh


def _prep_core(k, x, node_idx, hedge_idx, batch):
    s = int(np.searchsorted(batch, 2 * k))
    e = int(np.searchsorted(batch, 2 * k + 2))
    nk = e - s
    assert nk <= NK_PAD
    sel = np.where((node_idx >= s) & (node_idx < e))[0]
    na = (node_idx[sel] - s).astype(np.int64)
    ha = hedge_idx[sel].astype(np.int64)

    deg = np.bincount(node_idx, minlength=N).astype(np.float32)
    dloc = np.zeros(NK_PAD, np.float32)
    dl = deg[s:e]
    dloc[:nk] = np.where(dl > 0, 1.0 / np.maximum(dl, 1), 0.0)

    giA, ohA = _build_phase(ha, na, NWA, TW_A, WA)
    giB, ohB = _build_phase(na, ha, NWB, TW_B, WB, slot_scale=dloc)

    xT = np.zeros((F_IN, NK_PAD), np.float32)
    xT[:, :nk] = x[s:e].T

    p0 = np.zeros(NK_PAD, np.float32)
    p1 = np.zeros(NK_PAD, np.float32)
    bloc = batch[s:e]
    for gi, arr in ((2 * k, p0), (2 * k + 1, p1)):
        m = (bloc == gi)
        cnt = max(float(m.sum()), 1.0)
        arr[:nk][m] = 1.0 / cnt
    return {
        "xT": xT, "idxA": giA, "idxB": giB, "ohA": ohA, "ohB": ohB,
        "pool0_r": np.ascontiguousarray(
            np.broadcast_to(p0.astype(BF16), (128, NK_PAD))),
        "pool1_r": np.ascontiguousarray(
            np.broadcast_to(p1.astype(BF16), (128, NK_PAD))),
    }


def _shared_inputs(W0, b0, Wc1, bc1, Wc2, bc2, Wg, bg, W1, b1, g1, be1,
                   rm1, rv1, W2, b2, g2, be2, rm2, rv2, W3, hedge_idx):
    k1 = np.asarray(g1) / np.sqrt(np.asarray(rv1) + EPS)
    W1f = (np.asarray(W1) * k1[None, :]).astype(np.float32)
    b1f = ((np.asarray(b1) - np.asarray(rm1)) * k1 + np.asarray(be1)).astype(np.float32)
    k2 = np.asarray(g2) / np.sqrt(np.asarray(rv2) + EPS)
    W2f = (np.asarray(W2) * k2[None, :]).astype(np.float32)
    b2f = ((np.asarray(b2) - np.asarray(rm2)) * k2 + np.asarray(be2)).astype(np.float32)

    cnt = np.bincount(hedge_idx, minlength=M_PAD).astype(np.float32)
    binv = np.where(cnt > 0, 1.0 / np.maximum(cnt, 1), 0.0).astype(np.float32)

    Wg_np = np.asarray(Wg, np.float32)
    return {
        "binv_c": binv.reshape(-1, 128).T.copy(),
        "W0": np.asarray(W0, np.float32), "Wc1": np.asarray(Wc1, np.float32),
        "Wc2b": np.asarray(Wc2, np.float32).astype(BF16),
        "WgA": Wg_np[:H], "WgB": Wg_np[H:],
        "W1f": W1f, "W2f": W2f, "W3": np.asarray(W3, np.float32),
        "b0c": np.asarray(b0, np.float32).reshape(-1, 1),
        "bc1c": np.asarray(bc1, np.float32).reshape(-1, 1),
        "bc2c": np.asarray(bc2, np.float32).reshape(-1, 1),
        "bgc": np.asarray(bg, np.float32).reshape(-1, 1),
        "b1c": b1f.reshape(-1, 1), "b2c": b2f.reshape(-1, 1),
    }


def kernel(x, node_idx, hedge_idx, batch, W0, b0, Wc1, bc1, Wc2, bc2,
           Wg, bg, W1, b1, g1, be1, rm1, rv1, W2, b2, g2, be2, rm2, rv2, W3):
    global _COMPILED
    from concourse.bass_utils import run_bass_kernel_spmd

    x = np.asarray(x, np.float32)
    node_idx = np.asarray(node_idx).astype(np.int64)
    hedge_idx = np.asarray(hedge_idx).astype(np.int64)
    batch_np = np.asarray(batch).astype(np.int64)

    if _COMPILED is None:
        _COMPILED = _build_nc()
    nc = _COMPILED

    shared = _shared_inputs(W0, b0, Wc1, bc1, Wc2, bc2, Wg, bg, W1, b1, g1,
                            be1, rm1, rv1, W2, b2, g2, be2, rm2, rv2, W3,
                            hedge_idx)
    in_maps = []
    for k in range(NCORE):
        m = _prep_core(k, x, node_idx, hedge_idx, batch_np)
        m.update(shared)
        in_maps.append(m)

    r = run_bass_kernel_spmd(nc, in_maps, core_ids=list(range(NCORE)))
    out = np.zeros((B_GRAPHS, 4), np.float32)
    for k in range(NCORE):
        o = r.results[k]["out"]
        out[2 * k] = o[:, 0]
        out[2 * k + 1] = o[:, 1]
    return out


# revision 15
# speedup vs baseline: 1.9377x; 1.0170x over previous
"""Trainium2 Bass kernel for the hypergraph-conv survival model.

Graph/data parallel over 8 NeuronCores; core k owns graphs 2k,2k+1.
Both hconv directions are segment-sums done as one-hot matmuls; the
one-hot tiles are PRECOMPUTED ON THE HOST (bf16) and streamed from HBM
(they only depend on the static incidence lists), so no engine has to
generate them. Gather tables (z1/z2/eR) are bf16 to halve gather bytes.
Phase A uses 512-wide hedge windows (psum [128,512] f32, one bank);
phase B keeps 256-wide node windows feature-major for pooling + the
next-layer matmul. The AllReduce stays f32; a SWDGE cast-DMA converts
the reduced table to bf16 for the phase-B gathers. PSUM evacuation
copies run on the otherwise-idle Activation engine.
"""

import sys

sys.path.insert(0, "/opt/trn_rl_repo")

import numpy as np
import ml_dtypes

BF16 = ml_dtypes.bfloat16

# ---- static problem sizes (from the reference) ----
N = 100_000
E = 800_000
M = 25_000
B_GRAPHS = 16
F_IN = 64
H = 128
EPS = 1e-5
NCORE = 8

NK_PAD = 13312          # padded per-core node count (104*128)
M_PAD = 25088           # padded hyperedge count  (196*128)

WA = 512                # phase-A (hedge) window width
NWA = M_PAD // WA       # 49 windows
TW_A = 18               # tiles (128 edges) per hedge window (data max 2194)
E_PAD_A = NWA * TW_A * 128   # 112896
CHW_A = 3               # windows per gather chunk (last chunk = 1 window)

WB = 256                # phase-B (node) window width
NWB = NK_PAD // WB      # 52 windows
TW_B = 18               # tiles per node window (data max 2201)
E_PAD_B = NWB * TW_B * 128   # 119808
CHW_B = 4               # windows per gather chunk (13 chunks)
NCH_B = NWB // CHW_B

A_CHUNKS = [(i, CHW_A) for i in range(0, NWA - 1, CHW_A)] + [(NWA - 1, 1)]

_COMPILED = None


def _build_nc():
    import concourse.bacc as bacc
    import concourse.mybir as mybir
    from concourse.tile import TileContext
    from concourse import library_config

    f32 = mybir.dt.float32
    bf16 = mybir.dt.bfloat16
    i16 = mybir.dt.int16
    i32 = mybir.dt.int32
    EQ = mybir.AluOpType.is_equal
    ADD = mybir.AluOpType.add
    MAX = mybir.AluOpType.max
    RELU = mybir.ActivationFunctionType.Relu

    nc = bacc.Bacc("TRN2", target_bir_lowering=False, num_devices=NCORE,
                   num_swdge_queues=2)

    def inp(name, shape, dt=f32):
        return nc.dram_tensor(name, shape, dt, kind="ExternalInput")

    xT = inp("xT", [F_IN, NK_PAD], bf16)
    idxA = inp("idxA", [128, E_PAD_A // 16], i16)
    idxB = inp("idxB", [128, E_PAD_B // 16], i16)
    ohA_d = inp("ohA", [128, NWA * TW_A * WA], bf16)
    ohB_d = inp("ohB", [128, NWB * TW_B * WB], bf16)
    binv_c = inp("binv_c", [128, M_PAD // 128])
    pool0_r = inp("pool0_r", [128, NK_PAD], bf16)
    pool1_r = inp("pool1_r", [128, NK_PAD], bf16)
    W0_d = inp("W0", [F_IN, H], bf16)
    Wc1_d = inp("Wc1", [H, H], bf16)
    Wc2_d = inp("Wc2b", [H, H], bf16)
    WgA_d = inp("WgA", [H, H])
    WgB_d = inp("WgB", [H, H])
    W1_d = inp("W1f", [H, 64])
    W2_d = inp("W2f", [64, 32])
    W3_d = inp("W3", [32, 4])
    b0_d = inp("b0c", [H, 1])
    bc1_d = inp("bc1c", [H, 1])
    bc2_d = inp("bc2c", [H, 1])
    bg_d = inp("bgc", [H, 1])
    b1_d = inp("b1c", [64, 1])
    b2_d = inp("b2c", [32, 1])
    out_d = nc.dram_tensor("out", [4, 2], f32, kind="ExternalOutput")

    z1_h = nc.dram_tensor("z1_h", [NK_PAD, H], bf16)
    z2_h = nc.dram_tensor("z2_h", [NK_PAD, H], bf16)
    eA1 = nc.dram_tensor("eA1", [M_PAD, H], bf16)
    eR1 = nc.dram_tensor("eR1", [M_PAD, H], bf16, addr_space="Shared")
    eA2 = nc.dram_tensor("eA2", [M_PAD, H], bf16)
    eR2 = nc.dram_tensor("eR2", [M_PAD, H], bf16, addr_space="Shared")

    with TileContext(nc) as tc:
        with (
            tc.tile_pool(name="c", bufs=1) as cpool,
            tc.tile_pool(name="ga", bufs=2) as gapool,
            tc.tile_pool(name="gb", bufs=3) as gbpool,
            tc.tile_pool(name="oha", bufs=2) as ohapool,
            tc.tile_pool(name="ohb", bufs=2) as ohbpool,
            tc.tile_pool(name="s", bufs=3) as spool,
            tc.tile_pool(name="psA", bufs=2, space="PSUM") as psApool,
            tc.tile_pool(name="psB", bufs=2, space="PSUM") as psBpool,
            tc.tile_pool(name="psz", bufs=2, space="PSUM") as pszpool,
            tc.tile_pool(name="pst", bufs=2, space="PSUM") as pstpool,
            tc.tile_pool(name="acc", bufs=1) as accpool,
            tc.tile_pool(name="bc", bufs=1) as bcpool,
        ):
            nc.gpsimd.load_library(library_config.mlp)

            idn_i = cpool.tile([128, 128], i32)
            nc.gpsimd.iota(idn_i[:], [[1, 128]], channel_multiplier=-1)
            ident = cpool.tile([128, 128], f32)
            nc.vector.tensor_scalar(ident[:], idn_i[:], 0.0, None, EQ)

            def load_sb(dram, shape, dt=f32):
                t = cpool.tile(shape, dt, tag=dram.name + "_sb")
                nc.sync.dma_start(out=t[:], in_=dram[:, :])
                return t

            W0s = load_sb(W0_d, [F_IN, H], bf16)
            Wc1s = load_sb(Wc1_d, [H, H], bf16)
            Wc2s = load_sb(Wc2_d, [H, H], bf16)
            WgAs = load_sb(WgA_d, [H, H])
            WgBs = load_sb(WgB_d, [H, H])
            W1s = load_sb(W1_d, [H, 64])
            W2s = load_sb(W2_d, [64, 32])
            W3s = load_sb(W3_d, [32, 4])
            b0s = load_sb(b0_d, [H, 1])
            bc1s = load_sb(bc1_d, [H, 1])
            bc2s = load_sb(bc2_d, [H, 1])
            bgs = load_sb(bg_d, [H, 1])
            b1s = load_sb(b1_d, [64, 1])
            b2s = load_sb(b2_d, [32, 1])
            binvs = load_sb(binv_c, [128, M_PAD // 128])
            idxAs = load_sb(idxA, [128, E_PAD_A // 16], i16)
            idxBs = load_sb(idxB, [128, E_PAD_B // 16], i16)

            p1acc = accpool.tile([128, 2], f32)
            nc.vector.memset(p1acc[:], 0.0)
            p2acc = accpool.tile([128, 2], f32)
            nc.vector.memset(p2acc[:], 0.0)

            # ---- stage 1+2: z1 = (relu(x@W0+b0)) @ Wc1, bf16 rows to HBM ----
            for c in range(NK_PAD // 512):
                xc = spool.tile([F_IN, 512], bf16, tag="xc")
                nc.sync.dma_start(out=xc[:], in_=xT[:, c * 512:(c + 1) * 512])
                ps1 = pszpool.tile([128, 512], f32, tag="st1")
                nc.tensor.matmul(ps1[:], W0s[:], xc[:], start=True, stop=True)
                h1c = spool.tile([128, 512], bf16, tag="h1c")
                nc.scalar.activation(h1c[:], ps1[:], RELU, bias=b0s[:, 0:1])
                ps2 = pszpool.tile([128, 512], f32, tag="st1")
                nc.tensor.matmul(ps2[:], Wc1s[:], h1c[:], start=True, stop=True)
                zc = spool.tile([128, 512], f32, tag="zc1")
                nc.scalar.copy(zc[:], ps2[:])
                for h_ in range(4):
                    tp = pstpool.tile([128, 128], f32, tag="trp")
                    nc.tensor.transpose(tp[:], zc[:, h_ * 128:(h_ + 1) * 128], ident[:])
                    rows = spool.tile([128, 128], bf16, tag="rowsz")
                    nc.scalar.copy(rows[:], tp[:])
                    nc.scalar.dma_start(
                        out=z1_h[c * 512 + h_ * 128:c * 512 + (h_ + 1) * 128, :],
                        in_=rows[:])

            gq = [0]

            def next_q():
                gq[0] ^= 1
                return gq[0]

            # ---- conv phase A: partial hyperedge sums (512-wide windows) ----
            def phase_A(table_h, out_h):
                for w0, nw in A_CHUNKS:
                    ni = nw * TW_A * 128
                    g = gapool.tile([128, CHW_A * TW_A, 128], bf16, tag="gA")
                    o16 = w0 * TW_A * 128 // 16
                    nc.gpsimd.dma_gather(
                        g[:, :nw * TW_A, :], table_h.ap(),
                        idxAs[:, o16:o16 + ni // 16],
                        ni, ni, 128, single_packet=False, queue_num=next_q(),
                    )
                    for wi in range(nw):
                        w = w0 + wi
                        ohw = ohapool.tile([128, TW_A, WA], bf16, tag="ohA")
                        oc = w * TW_A * WA
                        nc.sync.dma_start(out=ohw[:], in_=ohA_d[:, oc:oc + TW_A * WA])
                        ps = psApool.tile([128, WA], f32, tag="pA")
                        for tt in range(TW_A):
                            nc.tensor.matmul(
                                ps[:], g[:, wi * TW_A + tt, :], ohw[:, tt, :],
                                start=(tt == 0), stop=(tt == TW_A - 1))
                        edc = spool.tile([128, WA], f32, tag="edcA")
                        nc.scalar.copy(edc[:], ps[:])
                        for j in range(WA // 128):
                            blk = w * (WA // 128) + j
                            tp = pstpool.tile([128, 128], f32, tag="trp")
                            nc.tensor.transpose(
                                tp[:], edc[:, j * 128:(j + 1) * 128], ident[:])
                            rows = spool.tile([128, 128], bf16, tag="rowsA")
                            nc.scalar.mul(rows[:], tp[:], binvs[:, blk:blk + 1])
                            nc.sync.dma_start(
                                out=out_h[blk * 128:(blk + 1) * 128, :], in_=rows[:])

            # ---- conv phase B: node sums + relu + pooling (+ next z) ----
            def phase_B(table_h, bias_s, pacc, Wnext, znext_h):
                for ch in range(NCH_B):
                    ni = CHW_B * TW_B * 128
                    g = gbpool.tile([128, CHW_B * TW_B, 128], bf16, tag="gB")
                    o16 = ch * ni // 16
                    nc.gpsimd.dma_gather(
                        g[:], table_h.ap(),
                        idxBs[:, o16:o16 + ni // 16],
                        ni, ni, 128, single_packet=False, queue_num=next_q(),
                    )
                    csl = slice(ch * CHW_B * WB, (ch + 1) * CHW_B * WB)
                    p0ch = bcpool.tile([128, CHW_B * WB], bf16, tag="p0ch")
                    nc.sync.dma_start(out=p0ch[:], in_=pool0_r[:, csl])
                    p1ch = bcpool.tile([128, CHW_B * WB], bf16, tag="p1ch")
                    nc.sync.dma_start(out=p1ch[:], in_=pool1_r[:, csl])
                    for wi in range(CHW_B):
                        w = ch * CHW_B + wi
                        ohw = ohbpool.tile([128, TW_B, WB], bf16, tag="ohB")
                        oc = w * TW_B * WB
                        nc.sync.dma_start(out=ohw[:], in_=ohB_d[:, oc:oc + TW_B * WB])
                        ps = psBpool.tile([128, WB], f32, tag="pB")
                        for tt in range(TW_B):
                            nc.tensor.matmul(
                                ps[:], g[:, wi * TW_B + tt, :], ohw[:, tt, :],
                                start=(tt == 0), stop=(tt == TW_B - 1))
                        sl = slice(wi * WB, (wi + 1) * WB)
                        h2c = spool.tile([128, WB], bf16, tag="h2c")
                        nc.vector.tensor_scalar(h2c[:], ps[:], bias_s[:], 0.0, ADD, MAX)
                        for gi, prow in ((0, p0ch), (1, p1ch)):
                            pm = spool.tile([128, WB], bf16, tag="pm")
                            nc.vector.tensor_mul(pm[:], h2c[:], prow[:, sl])
                            rs = spool.tile([128, 1], f32, tag="rs")
                            nc.vector.tensor_reduce(
                                rs[:], pm[:], mybir.AxisListType.X, ADD)
                            nc.vector.tensor_add(
                                pacc[:, gi:gi + 1], pacc[:, gi:gi + 1], rs[:])
                        if znext_h is not None:
                            zp_t = pszpool.tile([128, 512], f32, tag="st1")
                            zp = zp_t[:, 0:WB]
                            nc.tensor.matmul(zp, Wnext[:], h2c[:], start=True, stop=True)
                            zc = spool.tile([128, WB], f32, tag="zc")
                            nc.scalar.copy(zc[:], zp)
                            for h_ in range(2):
                                tp = pstpool.tile([128, 128], f32, tag="trp")
                                nc.tensor.transpose(
                                    tp[:], zc[:, h_ * 128:(h_ + 1) * 128], ident[:])
                                rows = spool.tile([128, 128], bf16, tag="rowsz")
                                nc.scalar.copy(rows[:], tp[:])
                                nc.scalar.dma_start(
                                    out=znext_h[w * WB + h_ * 128:w * WB + (h_ + 1) * 128, :],
                                    in_=rows[:])

            phase_A(z1_h, eA1)
            HM = M_PAD // 2
            nc.gpsimd.collective_compute(
                "AllReduce", ADD, replica_groups=[list(range(NCORE))],
                ins=[eA1[0:HM, :].opt()], outs=[eR1[0:HM, :].opt()])
            nc.gpsimd.collective_compute(
                "AllReduce", ADD, replica_groups=[list(range(NCORE))],
                ins=[eA1[HM:M_PAD, :].opt()], outs=[eR1[HM:M_PAD, :].opt()])
            phase_B(eR1, bc1s, p1acc, Wc2s, z2_h)
            phase_A(z2_h, eA2)
            nc.gpsimd.collective_compute(
                "AllReduce", ADD, replica_groups=[list(range(NCORE))],
                ins=[eA2[0:HM, :].opt()], outs=[eR2[0:HM, :].opt()])
            nc.gpsimd.collective_compute(
                "AllReduce", ADD, replica_groups=[list(range(NCORE))],
                ins=[eA2[HM:M_PAD, :].opt()], outs=[eR2[HM:M_PAD, :].opt()])
            phase_B(eR2, bc2s, p2acc, None, None)

            # ---- MLP head on the two pooled rows ----
            gps_t = psBpool.tile([128, WB], f32, tag="pB")
            gps = gps_t[:, 0:2]
            nc.tensor.matmul(gps, WgAs[:], p1acc[:], start=True, stop=False)
            nc.tensor.matmul(gps, WgBs[:], p2acc[:], start=False, stop=True)
            gb = spool.tile([128, 2], f32, tag="m1")
            nc.vector.tensor_scalar(gb[:], gps, bgs[:], None, ADD)
            h1ps = mlppool.tile([64, 2], f32, tag="mlp2")
            nc.tensor.matmul(h1ps[:], W1s[:], gb[:], start=True, stop=True)
            h1m = spool.tile([64, 2], f32, tag="m2")
            nc.vector.tensor_scalar(h1m[:], h1ps[:], b1s[:], 0.0, ADD, MAX)
            h2ps = mlppool.tile([32, 2], f32, tag="mlp3")
            nc.tensor.matmul(h2ps[:], W2s[:], h1m[:], start=True, stop=True)
            h2m = spool.tile([32, 2], f32, tag="m3")
            nc.vector.tensor_scalar(h2m[:], h2ps[:], b2s[:], 0.0, ADD, MAX)
            ops = mlppool.tile([4, 2], f32, tag="mlp4")
            nc.tensor.matmul(ops[:], W3s[:], h2m[:], start=True, stop=True)
            om = spool.tile([4, 2], f32, tag="m4")
            nc.vector.tensor_copy(om[:], ops[:])
            nc.sync.dma_start(out=out_d[:, :], in_=om[:])

    nc.compile()
    return nc


def _wrap_idx(idx):
    return np.tile(idx.reshape(-1, 16).T, (8, 1)).copy()


def _build_phase(keys, vals, nwin, tw, W, slot_scale=None):
    """Sorted segment layout: window-major slot list, idx + one-hot tiles.
    One-hot entries are 1.0, or slot_scale[key] when given (folds the
    per-destination normalization into the segment-sum matmul)."""
    order = np.argsort(keys, kind="stable")
    ks, vs = keys[order], vals[order]
    gidx = np.zeros(nwin * tw * 128, np.int64)
    widx = np.full(nwin * tw * 128, -1.0, np.float32)
    sval = np.zeros(nwin * tw * 128, np.float32)
    starts = np.searchsorted(ks, np.arange(nwin) * W)
    ends = np.searchsorted(ks, (np.arange(nwin) + 1) * W)
    scale = np.ones(len(ks), np.float32) if slot_scale is None \
        else slot_scale[ks].astype(np.float32)
    for w in range(nwin):
        a, b = starts[w], ends[w]
        n = b - a
        assert n <= tw * 128, f"window overflow {n} > {tw * 128}"
        o = w * tw * 128
        gidx[o:o + n] = vs[a:b]
        widx[o:o + n] = ks[a:b] - w * W
        sval[o:o + n] = scale[a:b]
    oh = (widx.reshape(nwin * tw, 128)[:, :, None] ==
          np.arange(W, dtype=np.float32)[None, None, :])
    oh = oh * sval.reshape(nwin * tw, 128)[:, :, None]
    oh = np.ascontiguousarray(
        oh.transpose(1, 0, 2).reshape(128, nwin * tw * W)).astype(BF16)
    return _wrap_idx(gidx.astype(np.int16)), oh


def _prep_core(k, x, node_idx, hedge_idx, batch):
    s = int(np.searchsorted(batch, 2 * k))
    e = int(np.searchsorted(batch, 2 * k + 2))
    nk = e - s
    assert nk <= NK_PAD
    sel = np.where((node_idx >= s) & (node_idx < e))[0]
    na = (node_idx[sel] - s).astype(np.int64)
    ha = hedge_idx[sel].astype(np.int64)

    deg = np.bincount(node_idx, minlength=N).astype(np.float32)
    dloc = np.zeros(NK_PAD, np.float32)
    dl = deg[s:e]
    dloc[:nk] = np.where(dl > 0, 1.0 / np.maximum(dl, 1), 0.0)

    giA, ohA = _build_phase(ha, na, NWA, TW_A, WA)
    giB, ohB = _build_phase(na, ha, NWB, TW_B, WB, slot_scale=dloc)

    xT = np.zeros((F_IN, NK_PAD), BF16)
    xT[:, :nk] = x[s:e].T.astype(BF16)

    p0 = np.zeros(NK_PAD, np.float32)
    p1 = np.zeros(NK_PAD, np.float32)
    bloc = batch[s:e]
    for gi, arr in ((2 * k, p0), (2 * k + 1, p1)):
        m = (bloc == gi)
        cnt = max(float(m.sum()), 1.0)
        arr[:nk][m] = 1.0 / cnt
    return {
        "xT": xT, "idxA": giA, "idxB": giB, "ohA": ohA, "ohB": ohB,
        "pool0_r": np.ascontiguousarray(
            np.broadcast_to(p0.astype(BF16), (128, NK_PAD))),
        "pool1_r": np.ascontiguousarray(
            np.broadcast_to(p1.astype(BF16), (128, NK_PAD))),
    }


def _shared_inputs(W0, b0, Wc1, bc1, Wc2, bc2, Wg, bg, W1, b1, g1, be1,
                   rm1, rv1, W2, b2, g2, be2, rm2, rv2, W3, hedge_idx):
    k1 = np.asarray(g1) / np.sqrt(np.asarray(rv1) + EPS)
    W1f = (np.asarray(W1) * k1[None, :]).astype(np.float32)
    b1f = ((np.asarray(b1) - np.asarray(rm1)) * k1 + np.asarray(be1)).astype(np.float32)
    k2 = np.asarray(g2) / np.sqrt(np.asarray(rv2) + EPS)
    W2f = (np.asarray(W2) * k2[None, :]).astype(np.float32)
    b2f = ((np.asarray(b2) - np.asarray(rm2)) * k2 + np.asarray(be2)).astype(np.float32)

    cnt = np.bincount(hedge_idx, minlength=M_PAD).astype(np.float32)
    binv = np.where(cnt > 0, 1.0 / np.maximum(cnt, 1), 0.0).astype(np.float32)

    Wg_np = np.asarray(Wg, np.float32)
    return {
        "binv_c": binv.reshape(-1, 128).T.copy(),
        "W0": np.asarray(W0, np.float32).astype(BF16),
        "Wc1": np.asarray(Wc1, np.float32).astype(BF16),
        "Wc2b": np.asarray(Wc2, np.float32).astype(BF16),
        "WgA": Wg_np[:H], "WgB": Wg_np[H:],
        "W1f": W1f, "W2f": W2f, "W3": np.asarray(W3, np.float32),
        "b0c": np.asarray(b0, np.float32).reshape(-1, 1),
        "bc1c": np.asarray(bc1, np.float32).reshape(-1, 1),
        "bc2c": np.asarray(bc2, np.float32).reshape(-1, 1),
        "bgc": np.asarray(bg, np.float32).reshape(-1, 1),
        "b1c": b1f.reshape(-1, 1), "b2c": b2f.reshape(-1, 1),
    }


def kernel(x, node_idx, hedge_idx, batch, W0, b0, Wc1, bc1, Wc2, bc2,
           Wg, bg, W1, b1, g1, be1, rm1, rv1, W2, b2, g2, be2, rm2, rv2, W3):
    global _COMPILED
    from concourse.bass_utils import run_bass_kernel_spmd

    x = np.asarray(x, np.float32)
    node_idx = np.asarray(node_idx).astype(np.int64)
    hedge_idx = np.asarray(hedge_idx).astype(np.int64)
    batch_np = np.asarray(batch).astype(np.int64)

    if _COMPILED is None:
        _COMPILED = _build_nc()
    nc = _COMPILED

    shared = _shared_inputs(W0, b0, Wc1, bc1, Wc2, bc2, Wg, bg, W1, b1, g1,
                            be1, rm1, rv1, W2, b2, g2, be2, rm2, rv2, W3,
                            hedge_idx)
    in_maps = []
    for k in range(NCORE):
        m = _prep_core(k, x, node_idx, hedge_idx, batch_np)
        m.update(shared)
        in_maps.append(m)

    r = run_bass_kernel_spmd(nc, in_maps, core_ids=list(range(NCORE)))
    out = np.zeros((B_GRAPHS, 4), np.float32)
    for k in range(NCORE):
        o = r.results[k]["out"]
        out[2 * k] = o[:, 0]
        out[2 * k + 1] = o[:, 1]
    return out
